# revision 6
# baseline (speedup 1.0000x reference)
"""Trainium2 Bass kernel for nn_CrossAttentionFusion (B=16384, D=2048, fp32).

Math: in the reference, softmax is taken over a length-1 axis, so it is
identically 1.0 and the q/k projections are dead code:

    out = (emb_b @ Wv.T + bv + emb_a) @ Wo.T + bo
        = emb_b @ (Wv.T @ Wo.T) + emb_a @ Wo.T + (Wo @ bv + bo)

The kernel computes the fused two-matmul form.  Host precomputes
Wc = Wv.T @ Wo.T (fp32) and bias_c = Wo @ bv + bo; both output-feature
contributions accumulate in PSUM on-chip.

Sharding: data-parallel over the batch dim, 2048 rows per NeuronCore.
Layout is feature-major on-device (features on partitions, rows on the
free dim), so no transposes are ever needed on-device; the host
transposes the embedding shards in and the output shards back out.

Numerics: matmul operands are cast to bf16 on host (PE array runs bf16
at 1 cycle/row vs 4 for fp32); accumulation is fp32 in PSUM.
"""

import numpy as np
import ml_dtypes

import concourse.bass as bass
import concourse.mybir as mybir
import concourse.tile as tile
from concourse import bacc
from concourse.bass import ts
from concourse.bass_utils import run_bass_kernel_spmd

BF16 = ml_dtypes.bfloat16

NCORES = 8
B = 16384
D = 2048
R = B // NCORES          # rows per core
P = 128                  # partitions
KO = D // P              # contraction chunks (16)
MO = D // P              # output-feature chunks (16)
NT = 512                 # rows per matmul (moving free dim)
NB = R // NT             # row blocks per core (4)

_NC_CACHE = {}

# Exposed for test harnesses: BassKernelResults of the most recent run.
LAST_RESULT = None


def _build_bass(D=D, R=R, NT=NT):
    """Per-core program: outt[D, R] = Wc.T-path(ebt) + WoT-path(eat) + bias."""
    KO = D // P
    MO = D // P
    NB = R // NT
    # Bacc (not raw Bass): its compile() splits multi-sem waits into
    # InstEventSemaphore (TRN2 allows at most one sync wait per instruction).
    nc = bacc.Bacc(None, target_bir_lowering=False)
    f32 = mybir.dt.float32
    bf16 = mybir.dt.bfloat16

    ebt_d = nc.dram_tensor("ebt", [D, R], bf16, kind="ExternalInput")
    eat_d = nc.dram_tensor("eat", [D, R], bf16, kind="ExternalInput")
    wc_d = nc.dram_tensor("wc", [D, D], bf16, kind="ExternalInput")
    wot_d = nc.dram_tensor("wot", [D, D], bf16, kind="ExternalInput")
    bias_d = nc.dram_tensor("bias", [D], f32, kind="ExternalInput")
    outt_d = nc.dram_tensor("outt", [D, R], f32, kind="ExternalOutput")

    ebt_r = ebt_d.rearrange("(ko p) r -> p ko r", p=P)
    eat_r = eat_d.rearrange("(ko p) r -> p ko r", p=P)
    wc_r = wc_d.rearrange("(ko p) m -> p ko m", p=P)
    wot_r = wot_d.rearrange("(ko p) m -> p ko m", p=P)
    bias_r = bias_d.rearrange("(mo p) -> p mo", p=P)

    with tile.TileContext(nc) as tc:
        with (
            tc.tile_pool(name="weights", bufs=1) as wpool,
            tc.tile_pool(name="acts", bufs=2) as apool,
            tc.tile_pool(name="outs", bufs=4) as opool,
            tc.tile_pool(name="psum", bufs=8, space="PSUM") as pspool,
        ):
            wc_sb = wpool.tile([P, KO, D], bf16, tag="wc")
            wot_sb = wpool.tile([P, KO, D], bf16, tag="wot")
            bias_st = wpool.tile([P, MO], f32, tag="bias_st")
            bias_sb = wpool.tile([P, MO], f32, tag="bias")

            # Stage bias through a DVE copy: the per-tile bias-add TensorTensor
            # then depends only on PE (TT has a single HW sync-wait slot).
            nc.sync.dma_start(bias_st[:], bias_r[:])
            nc.vector.tensor_copy(bias_sb[:], bias_st[:])
            for ko in range(KO):
                nc.sync.dma_start(wc_sb[:, ko, :], wc_r[:, ko, :])
                nc.sync.dma_start(wot_sb[:, ko, :], wot_r[:, ko, :])

            for nb in range(NB):
                eb_t = apool.tile([P, KO, NT], bf16, tag="eb")
                ea_t = apool.tile([P, KO, NT], bf16, tag="ea")
                for ko in range(KO):
                    nc.sync.dma_start(eb_t[:, ko, :], ebt_r[:, ko, ts(nb, NT)])
                    nc.sync.dma_start(ea_t[:, ko, :], eat_r[:, ko, ts(nb, NT)])

                for mo in range(MO):
                    ps = pspool.tile([P, NT], f32, tag="ps")
                    for ko in range(KO):
                        nc.tensor.matmul(
                            ps[:],
                            wc_sb[:, ko, ts(mo, P)],
                            eb_t[:, ko, :],
                            start=(ko == 0),
                            stop=False,
                        )
                    for ko in range(KO):
                        nc.tensor.matmul(
                            ps[:],
                            wot_sb[:, ko, ts(mo, P)],
                            ea_t[:, ko, :],
                            start=False,
                            stop=(ko == KO - 1),
                        )
                    ot = opool.tile([P, NT], f32, tag="ot")
                    nc.vector.tensor_tensor(
                        ot[:],
                        ps[:],
                        bias_sb[:, mo : mo + 1].to_broadcast((P, NT)),
                        mybir.AluOpType.add,
                    )
                    nc.sync.dma_start(outt_d[ts(mo, P), ts(nb, NT)], ot[:])

    nc.compile()
    return nc


def kernel(emb_a, emb_b, Wq, bq, Wk, bk, Wv, bv, Wo, bo):
    global LAST_RESULT
    emb_a = np.asarray(emb_a, dtype=np.float32)
    emb_b = np.asarray(emb_b, dtype=np.float32)
    Wv = np.asarray(Wv, dtype=np.float32)
    bv = np.asarray(bv, dtype=np.float32)
    Wo = np.asarray(Wo, dtype=np.float32)
    bo = np.asarray(bo, dtype=np.float32)

    # Fused weights / bias (q/k are dead code: softmax over a length-1
    # axis is exactly 1.0).
    Wc = np.matmul(Wv.T, Wo.T)                       # [D_in, D_out] fp32
    bias = (Wo.astype(np.float64) @ bv.astype(np.float64) + bo).astype(np.float32)

    wc_bf = Wc.astype(BF16)
    wot_bf = Wo.T.astype(BF16, order="C")

    ea_bf = emb_a.astype(BF16)
    eb_bf = emb_b.astype(BF16)

    in_maps = []
    for c in range(NCORES):
        sl = slice(c * R, (c + 1) * R)
        in_maps.append(
            {
                "ebt": np.ascontiguousarray(eb_bf[sl].T),
                "eat": np.ascontiguousarray(ea_bf[sl].T),
                "wc": wc_bf,
                "wot": wot_bf,
                "bias": bias,
            }
        )

    if "nc" not in _NC_CACHE:
        _NC_CACHE["nc"] = _build_bass()
    nc = _NC_CACHE["nc"]

    res = run_bass_kernel_spmd(nc, in_maps, core_ids=list(range(NCORES)))
    LAST_RESULT = res

    out = np.empty((B, D), dtype=np.float32)
    for c in range(NCORES):
        out[c * R : (c + 1) * R, :] = res.results[c]["outt"].T
    return out


# revision 7
# speedup vs baseline: 23786.1426x; 23786.1426x over previous
"""Trainium2 Bass kernel for nn_CrossAttentionFusion (B=16384, D=2048, fp32).

Math: in the reference, softmax is taken over a length-1 axis, so it is
identically 1.0 and the q/k projections are dead code:

    out = (emb_b @ Wv.T + bv + emb_a) @ Wo.T + bo
        = emb_b @ (Wv.T @ Wo.T) + emb_a @ Wo.T + (Wo @ bv + bo)

The kernel computes the fused two-matmul form.  Host precomputes
Wc = Wv.T @ Wo.T (fp32) and bias_c = Wo @ bv + bo; both output-feature
contributions accumulate in PSUM on-chip.

Sharding: data-parallel over the batch dim, 2048 rows per NeuronCore.
Layout is feature-major on-device (features on partitions, rows on the
free dim), so no transposes are ever needed on-device; the host
transposes the embedding shards in and the output shards back out.

Numerics: matmul operands are cast to bf16 on host (PE array runs bf16
at 1 cycle/row vs 4 for fp32); accumulation is fp32 in PSUM.
"""

import numpy as np
import ml_dtypes

import concourse.bass as bass
import concourse.mybir as mybir
import concourse.tile as tile
from concourse import bacc
from concourse.bass import ts
from concourse.bass_utils import run_bass_kernel_spmd

BF16 = ml_dtypes.bfloat16

NCORES = 8
B = 16384
D = 2048
R = B // NCORES          # rows per core
P = 128                  # partitions
KO = D // P              # contraction chunks (16)
MO = D // P              # output-feature chunks (16)
NT = 512                 # rows per matmul (moving free dim)
NB = R // NT             # row blocks per core (4)

_NC_CACHE = {}

# Exposed for test harnesses: BassKernelResults of the most recent run.
LAST_RESULT = None


def _build_bass(D=D, R=R, NT=NT):
    """Per-core program: outt[D, R] = Wc.T-path(ebt) + WoT-path(eat) + bias."""
    KO = D // P
    MO = D // P
    NB = R // NT
    # Bacc (not raw Bass): its compile() splits multi-sem waits into
    # InstEventSemaphore (TRN2 allows at most one sync wait per instruction).
    nc = bacc.Bacc(None, target_bir_lowering=False)
    f32 = mybir.dt.float32
    bf16 = mybir.dt.bfloat16

    ebt_d = nc.dram_tensor("ebt", [D, R], bf16, kind="ExternalInput")
    eat_d = nc.dram_tensor("eat", [D, R], bf16, kind="ExternalInput")
    wc_d = nc.dram_tensor("wc", [D, D], bf16, kind="ExternalInput")
    wot_d = nc.dram_tensor("wot", [D, D], bf16, kind="ExternalInput")
    bias_d = nc.dram_tensor("bias", [D], f32, kind="ExternalInput")
    outt_d = nc.dram_tensor("outt", [D, R], f32, kind="ExternalOutput")

    ebt_r = ebt_d.rearrange("(ko p) r -> p ko r", p=P)
    eat_r = eat_d.rearrange("(ko p) r -> p ko r", p=P)
    wc_r = wc_d.rearrange("(ko p) m -> p ko m", p=P)
    wot_r = wot_d.rearrange("(ko p) m -> p ko m", p=P)
    bias_r = bias_d.rearrange("(mo p) -> p mo", p=P)

    with tile.TileContext(nc) as tc:
        with (
            tc.tile_pool(name="weights", bufs=1) as wpool,
            tc.tile_pool(name="acts", bufs=2) as apool,
            tc.tile_pool(name="outs", bufs=4) as opool,
            tc.tile_pool(name="psum", bufs=8, space="PSUM") as pspool,
        ):
            wc_sb = wpool.tile([P, KO, D], bf16, tag="wc")
            wot_sb = wpool.tile([P, KO, D], bf16, tag="wot")
            bias_st = wpool.tile([P, MO], f32, tag="bias_st")
            bias_sb = wpool.tile([P, MO], f32, tag="bias")

            # Stage bias through a DVE copy: the per-tile bias-add TensorTensor
            # then depends only on PE (TT has a single HW sync-wait slot).
            nc.sync.dma_start(bias_st[:], bias_r[:])
            nc.vector.tensor_copy(bias_sb[:], bias_st[:])

            # Weights stream in m-column blocks so the first matmul group only
            # gates on ~2MB (first block of each matrix + first act block), not
            # the full 16MB; remaining blocks overlap compute.
            MB = 8
            MBW = D // MB

            def load_w_block(mb):
                sl = ts(mb, MBW)
                nc.sync.dma_start(wc_sb[:, :, sl], wc_r[:, :, sl])
                nc.sync.dma_start(wot_sb[:, :, sl], wot_r[:, :, sl])

            def load_acts(nb):
                eb_t = apool.tile([P, KO, NT], bf16, tag="eb")
                ea_t = apool.tile([P, KO, NT], bf16, tag="ea")
                for ko in range(KO):
                    nc.sync.dma_start(eb_t[:, ko, :], ebt_r[:, ko, ts(nb, NT)])
                    nc.sync.dma_start(ea_t[:, ko, :], eat_r[:, ko, ts(nb, NT)])
                return eb_t, ea_t

            load_w_block(0)
            acts0 = load_acts(0)
            for mb in range(1, MB):
                load_w_block(mb)

            for nb in range(NB):
                eb_t, ea_t = acts0 if nb == 0 else load_acts(nb)

                for mo in range(MO):
                    ps = pspool.tile([P, NT], f32, tag="ps")
                    for ko in range(KO):
                        nc.tensor.matmul(
                            ps[:],
                            wc_sb[:, ko, ts(mo, P)],
                            eb_t[:, ko, :],
                            start=(ko == 0),
                            stop=False,
                        )
                    for ko in range(KO):
                        nc.tensor.matmul(
                            ps[:],
                            wot_sb[:, ko, ts(mo, P)],
                            ea_t[:, ko, :],
                            start=False,
                            stop=(ko == KO - 1),
                        )
                    ot = opool.tile([P, NT], f32, tag="ot")
                    nc.vector.tensor_tensor(
                        ot[:],
                        ps[:],
                        bias_sb[:, mo : mo + 1].to_broadcast((P, NT)),
                        mybir.AluOpType.add,
                    )
                    nc.sync.dma_start(outt_d[ts(mo, P), ts(nb, NT)], ot[:])

    nc.compile()
    return nc


def kernel(emb_a, emb_b, Wq, bq, Wk, bk, Wv, bv, Wo, bo):
    global LAST_RESULT
    emb_a = np.asarray(emb_a, dtype=np.float32)
    emb_b = np.asarray(emb_b, dtype=np.float32)
    Wv = np.asarray(Wv, dtype=np.float32)
    bv = np.asarray(bv, dtype=np.float32)
    Wo = np.asarray(Wo, dtype=np.float32)
    bo = np.asarray(bo, dtype=np.float32)

    # Fused weights / bias (q/k are dead code: softmax over a length-1
    # axis is exactly 1.0).
    Wc = np.matmul(Wv.T, Wo.T)                       # [D_in, D_out] fp32
    bias = (Wo.astype(np.float64) @ bv.astype(np.float64) + bo).astype(np.float32)

    wc_bf = Wc.astype(BF16)
    wot_bf = Wo.T.astype(BF16, order="C")

    ea_bf = emb_a.astype(BF16)
    eb_bf = emb_b.astype(BF16)

    in_maps = []
    for c in range(NCORES):
        sl = slice(c * R, (c + 1) * R)
        in_maps.append(
            {
                "ebt": np.ascontiguousarray(eb_bf[sl].T),
                "eat": np.ascontiguousarray(ea_bf[sl].T),
                "wc": wc_bf,
                "wot": wot_bf,
                "bias": bias,
            }
        )

    if "nc" not in _NC_CACHE:
        _NC_CACHE["nc"] = _build_bass()
    nc = _NC_CACHE["nc"]

    res = run_bass_kernel_spmd(nc, in_maps, core_ids=list(range(NCORES)))
    LAST_RESULT = res

    out = np.empty((B, D), dtype=np.float32)
    for c in range(NCORES):
        out[c * R : (c + 1) * R, :] = res.results[c]["outt"].T
    return out
